# revision 2
# baseline (speedup 1.0000x reference)
"""Periodic-boundary fixed-capacity neighbour list on 8 trn2 NeuronCores.

Algorithm (device, per core, rows sharded 256/core as 2 partition-tiles):
  For unit cell + cutoff 0.3, a pair (i, j) can be within cutoff for at most
  ONE of the 27 periodic images (per-axis shift intervals are disjoint), so
  the N x 27N reference mask collapses to N x N with a computed image index:
     g_a = -round(p_j,a - p_i,a)  in {-1,0,1}
     w_a = (g_a + p_j,a) - p_i,a      (fp32, reference-exact op order)
     hit = ((wx^2 + wy^2) + wz^2) <= 0.09f  and  j != i
     key = ((13 + gx + 3 gy + 9 gz) * 2048 + j)  (fp32-exact, < 2^24)
  argwhere packing == ascending-key order == take 256 smallest keys sorted.
  That selection runs as an oblivious bitonic top-256: sort eight 256-chunks
  ascending, then 3 rounds of reversed-compare prune-merges (8->4->2->1).
Host: shard/replicate inputs, decode keys -> neighbours/cell_indices, max
of per-row hit counts -> actual_max. jnp.take(idx=-1) wraps, so invalid
cell_indices slots are shifts[26] = (1,1,1).
"""
import sys

if '/opt/trn_rl_repo' not in sys.path:
    sys.path.insert(0, '/opt/trn_rl_repo')

import numpy as np

N = 2048
K = 256
CHUNK = 256
NCORES = 8
ROWS_PER_CORE = N // NCORES  # 256
NTILES = ROWS_PER_CORE // 128  # 2
BIG = 1.0e9
C_BASE = 13 * 2048  # 26624
THR = 0.3 * 0.3  # fp32-converts to 0.090000004 like the jax reference

_cached = {}


def _build_program():
    import concourse.bacc as bacc
    import concourse.mybir as mybir
    from concourse.tile import TileContext

    f32 = mybir.dt.float32
    i32 = mybir.dt.int32
    u32 = mybir.dt.uint32
    Alu = mybir.AluOpType
    Ax = mybir.AxisListType

    nc = bacc.Bacc("TRN2", target_bir_lowering=False)

    pjb_d = nc.dram_tensor("pjb", [3, N], f32, kind="ExternalInput")
    pit_d = nc.dram_tensor("pit", [NTILES, 128, 3], f32, kind="ExternalInput")
    ig_d = nc.dram_tensor("ig", [NTILES, 128, 1], f32, kind="ExternalInput")
    keys_d = nc.dram_tensor("keys", [NTILES, 128, K], f32, kind="ExternalOutput")
    cnt_d = nc.dram_tensor("cnt", [NTILES, 128, 1], f32, kind="ExternalOutput")

    with TileContext(nc) as tc:
        with tc.tile_pool(name="main", bufs=1) as pool:
            big = [128, N]
            pj = [pool.tile(big, f32, name=f"pj{a}", tag=f"pj{a}") for a in range(3)]
            iota_i = pool.tile(big, i32, tag="iotai")
            iota_f = pool.tile(big, f32, tag="iotaf")

            for a in range(3):
                nc.sync.dma_start(
                    out=pj[a][:],
                    in_=pjb_d[a:a + 1, :].partition_broadcast(128).squeeze(1),
                )
            nc.gpsimd.iota(iota_i[:], pattern=[[1, N]], base=C_BASE,
                           channel_multiplier=0)
            nc.vector.tensor_copy(out=iota_f[:], in_=iota_i[:])

            for t in range(NTILES):
                pit_s = pool.tile([128, 3], f32, tag="pit")
                ig_s = pool.tile([128, 1], f32, tag="ig")
                hi_s = pool.tile([128, 3], f32, tag="hi")
                lo_s = pool.tile([128, 3], f32, tag="lo")
                cnt_s = pool.tile([128, 1], f32, tag="cnt")
                nc.sync.dma_start(out=pit_s[:], in_=pit_d[t])
                nc.sync.dma_start(out=ig_s[:], in_=ig_d[t])
                nc.vector.tensor_scalar(out=hi_s[:], in0=pit_s[:], scalar1=0.5,
                                        scalar2=None, op0=Alu.add)
                nc.vector.tensor_scalar(out=lo_s[:], in0=pit_s[:], scalar1=-0.5,
                                        scalar2=None, op0=Alu.add)

                s = [pool.tile(big, f32, name=f"s{a}", tag=f"s{a}") for a in range(3)]
                w = [pool.tile(big, f32, name=f"w{a}", tag=f"w{a}") for a in range(3)]
                t1 = pool.tile(big, f32, tag="t1")
                t2 = pool.tile(big, f32, tag="t2")
                d2 = pool.tile(big, f32, tag="d2")
                hitm = pool.tile(big, f32, tag="hitm")
                hitu = pool.tile(big, u32, tag="iotai")  # reuse iota_i slot

                for a in range(3):
                    # g = (pj < pi-0.5) - (pj > pi+0.5)
                    nc.vector.tensor_scalar(out=t1[:], in0=pj[a][:],
                                            scalar1=hi_s[:, a:a + 1],
                                            scalar2=None, op0=Alu.is_gt)
                    nc.vector.tensor_scalar(out=t2[:], in0=pj[a][:],
                                            scalar1=lo_s[:, a:a + 1],
                                            scalar2=None, op0=Alu.is_lt)
                    nc.vector.tensor_tensor(out=s[a][:], in0=t2[:], in1=t1[:],
                                            op=Alu.subtract)
                    # u = g + pj ; w = u - pi   (reference-exact order)
                    nc.vector.tensor_tensor(out=t1[:], in0=s[a][:], in1=pj[a][:],
                                            op=Alu.add)
                    nc.vector.tensor_scalar(out=w[a][:], in0=t1[:],
                                            scalar1=pit_s[:, a:a + 1],
                                            scalar2=None, op0=Alu.subtract)

                # d2 = (wx*wx + wy*wy) + wz*wz
                nc.vector.tensor_tensor(out=d2[:], in0=w[0][:], in1=w[0][:],
                                        op=Alu.mult)
                nc.vector.tensor_tensor(out=t1[:], in0=w[1][:], in1=w[1][:],
                                        op=Alu.mult)
                nc.vector.tensor_tensor(out=t2[:], in0=d2[:], in1=t1[:],
                                        op=Alu.add)
                nc.vector.tensor_tensor(out=t1[:], in0=w[2][:], in1=w[2][:],
                                        op=Alu.mult)
                nc.vector.tensor_tensor(out=d2[:], in0=t2[:], in1=t1[:],
                                        op=Alu.add)

                nc.vector.tensor_scalar(out=t1[:], in0=d2[:], scalar1=THR,
                                        scalar2=None, op0=Alu.is_le)
                nc.vector.tensor_scalar(out=t2[:], in0=iota_f[:],
                                        scalar1=ig_s[:], scalar2=None,
                                        op0=Alu.not_equal)
                nc.vector.tensor_tensor(out=hitm[:], in0=t1[:], in1=t2[:],
                                        op=Alu.mult)
                nc.vector.tensor_reduce(out=cnt_s[:], in_=hitm[:], axis=Ax.X,
                                        op=Alu.add)
                nc.sync.dma_start(out=cnt_d[t], in_=cnt_s[:])
                nc.vector.tensor_scalar(out=hitu[:], in0=hitm[:], scalar1=0.0,
                                        scalar2=None, op0=Alu.is_gt)

                # key = ((gx + 3 gy + 9 gz) * 2048) + (26624 + j)
                k1 = pool.tile(big, f32, tag="w0")
                k2 = pool.tile(big, f32, tag="w1")
                k3 = pool.tile(big, f32, tag="w2")
                nc.vector.scalar_tensor_tensor(out=k1[:], in0=s[1][:], scalar=3.0,
                                               in1=s[0][:], op0=Alu.mult,
                                               op1=Alu.add)
                nc.vector.scalar_tensor_tensor(out=k2[:], in0=s[2][:], scalar=9.0,
                                               in1=k1[:], op0=Alu.mult,
                                               op1=Alu.add)
                nc.vector.scalar_tensor_tensor(out=k3[:], in0=k2[:], scalar=2048.0,
                                               in1=iota_f[:], op0=Alu.mult,
                                               op1=Alu.add)

                A = pool.tile(big, f32, tag="A")
                B = pool.tile(big, f32, tag="B")
                nc.gpsimd.memset(A[:], BIG)
                nc.vector.copy_predicated(A[:], hitu[:], k3[:])

                # ---- phase 2: bitonic top-256 ----
                cur, other = A, B

                def substage(lo_in, hi_in, lo_out, hi_out):
                    nonlocal cur, other
                    nc.vector.tensor_tensor(out=lo_out, in0=lo_in, in1=hi_in,
                                            op=Alu.min)
                    nc.vector.tensor_tensor(out=hi_out, in0=lo_in, in1=hi_in,
                                            op=Alu.max)
                    cur, other = other, cur

                def dist_substage(width, d):
                    r_in = cur[:, :width].rearrange("p (b r) -> p b r", r=2 * d)
                    r_out = other[:, :width].rearrange("p (b r) -> p b r", r=2 * d)
                    substage(r_in[:, :, 0:d], r_in[:, :, d:2 * d],
                             r_out[:, :, 0:d], r_out[:, :, d:2 * d])

                # stage A: sort each 256-chunk ascending
                for mexp in range(8):
                    m = 1 << mexp
                    r_in = cur[:].rearrange("p (b r) -> p b r", r=2 * m)
                    r_out = other[:].rearrange("p (b r) -> p b r", r=2 * m)
                    substage(r_in[:, :, 0:m], r_in[:, :, m:2 * m][:, :, ::-1],
                             r_out[:, :, 0:m], r_out[:, :, m:2 * m][:, :, ::-1])
                    d = m // 2
                    while d >= 1:
                        dist_substage(N, d)
                        d //= 2

                # stage B: prune-merges 8 -> 4 -> 2 -> 1 lists of 256
                width = N
                while width > CHUNK:
                    half = width // 2
                    r_in = cur[:, :width].rearrange("p (l r) -> p l r",
                                                    r=2 * CHUNK)
                    r_out = other[:, :half].rearrange("p (l r) -> p l r",
                                                      r=CHUNK)
                    nc.vector.tensor_tensor(
                        out=r_out[:],
                        in0=r_in[:, :, 0:CHUNK],
                        in1=r_in[:, :, CHUNK:2 * CHUNK][:, :, ::-1],
                        op=Alu.min)
                    cur, other = other, cur
                    d = CHUNK // 2
                    while d >= 1:
                        dist_substage(half, d)
                        d //= 2
                    width = half

                nc.sync.dma_start(out=keys_d[t], in_=cur[:, :K])

    nc.compile()
    return nc


def _get_program():
    if "nc" not in _cached:
        _cached["nc"] = _build_program()
    return _cached["nc"]


def kernel(positions, cell, max_neighbours):
    from concourse.bass_utils import run_bass_kernel_spmd

    pos = np.asarray(positions, dtype=np.float32)
    assert pos.shape == (N, 3)
    k = int(max_neighbours)
    assert k == K, f"kernel hardcodes max_neighbours=256, got {k}"

    nc = _get_program()

    pjb = np.ascontiguousarray(pos.T)  # [3, N]
    in_maps = []
    for cr in range(NCORES):
        rows0 = cr * ROWS_PER_CORE
        pit = pos[rows0: rows0 + ROWS_PER_CORE].reshape(NTILES, 128, 3)
        ig = (C_BASE + rows0 + np.arange(ROWS_PER_CORE, dtype=np.float32)
              ).reshape(NTILES, 128, 1).astype(np.float32)
        in_maps.append({
            "pjb": pjb,
            "pit": np.ascontiguousarray(pit),
            "ig": np.ascontiguousarray(ig),
        })

    res = run_bass_kernel_spmd(nc, in_maps, core_ids=list(range(NCORES)))

    keys = np.concatenate(
        [r["keys"].reshape(ROWS_PER_CORE, K) for r in res.results], axis=0)
    counts = np.concatenate(
        [r["cnt"].reshape(ROWS_PER_CORE) for r in res.results], axis=0)

    valid = keys < BIG / 2
    ki = np.round(keys).astype(np.int64)
    j = ki & (N - 1)
    c = ki >> 11
    neighbours = np.where(valid, j, -1).astype(np.int32)
    sx = c % 3 - 1
    sy = (c // 3) % 3 - 1
    sz = c // 9 - 1
    cells = np.stack([sx, sy, sz], axis=-1)
    cells = np.where(valid[..., None], cells, 1).astype(np.int32)
    actual_max = np.int32(counts.max())
    return neighbours, cells, actual_max
